# revision 48
# baseline (speedup 1.0000x reference)
"""ConvDeepSet Bass kernel for Trainium2 (8 NeuronCores, data-parallel over batch).

Math per batch b (reference):
    d[n,m]   = (x[n] - t[m])^2
    wt[n,m,c]= exp(-0.5 * d[n,m] / exp(sigma[c])^2) = exp(-alpha_c * d[n,m])
    ycat     = [ones, y]                       # (N, 9)
    yout[m,c]= sum_n ycat[n,c] * wt[n,m,c]     # (M, 9)
    h        = [yout[:,0], yout[:,1:]/(yout[:,0:1]+1e-8)]
    out      = h @ W + b                       # (M, 64)

Device mapping (one batch per core, n on partitions):
  Prologue: t broadcast to all 128 partitions via gpsimd partition_broadcast
           (much cheaper than a DMA broadcast); all small operands arrive
           packed in two host-prepared arrays (one fp32, one fp16) = 2 DMAs.
  Stage A+B ('derf' mode): ONE ACT pass per n-tile computes the Gaussian
           directly via Derivative_Erf(z) = (2/sqrt(pi)) * exp(-z^2) with
           z = sqrt(alpha)*t - sqrt(alpha)*x (free scale/bias of the
           activation); the sqrt(pi)/2 constant is folded into ycat
           host-side. The exp argument stays fp32 (precision-critical); the
           Gaussian values are emitted as fp16 (insensitive). The
           n-contraction runs as fp16 matmuls with ycat as lhsT accumulating
           y_out[9, M] in fp32 PSUM (channels permuted, density last).
  Stage C (per m-half, pipelined): y_out copied from PSUM to SBUF as fp16
           (one half on ACT, one on DVE). m-tile jj covers m = h*1024+8p+jj
           so each partition's 8 output rows are 2KB-contiguous in DRAM.
           Per tile ONE merged fp16 matmul with rhs [9, 128] =
           [W_conv*W0[o*] | E-block] writes [A' | E] into packed PSUM
           (4 tiles per 2KB bank, seeded per-bank with the bias pattern by a
           K=1 ones-row matmul). E = b + D*W0 with b[o*] zeroed, so
           r = 1/E[o*] recovers 1/(density*W0[o*]) and the DVE epilogue is
           just out = A'*r + E (reference's +1e-8 is a fp32 no-op since
           density is O(10)). One output DMA per half, on separate queues.

sigma is read on the host; channels are grouped by unique alpha (the
reference setup always produces a single group, the fast path; grouped
fallback handles arbitrary sigma). Degenerate all-zero W[0] falls back to a
full-fp32 stage C. Compiled programs are cached per grouping/config.

Measured (8-core SPMD, hardware loop differencing): ~25us per full kernel
iteration incl. setup DMAs; absmax/scale vs reference = 5.9e-4 (fp16
stage-C), 9.9e-5 with C_FP16=False.
"""

import contextlib

import numpy as np

import concourse.bass as bass
import concourse.mybir as mybir
import concourse.tile as tile
from concourse import bacc
from concourse.bass_utils import run_bass_kernel_spmd

B, N, M = 8, 512, 2048
JT0 = 8              # m-tiles per half (stage C)
CIN = 8
C = CIN + 1          # 9 channels incl. density
O = 64
P = 128
NT = N // P          # 4 n-tiles
MT = M // P          # 16 m-tiles for stage C
F32 = mybir.dt.float32
F32R = mybir.dt.float32r
BF16 = mybir.dt.bfloat16
FP16 = mybir.dt.float16
AF = mybir.ActivationFunctionType

_PROG_CACHE: dict = {}


def _e_dt():
    return {"bf16": BF16, "fp16": FP16}.get(E_DTYPE, F32R)


def _aux_layout(G):
    """Column offsets in the packed fp32 aux array: negx | w0b | bb | wext.
    The wext region holds G blocks of (O+1) columns, each block's rows at
    partitions [0:ng] (base partition 0 for every group's matmul rhs)."""
    negx_ofs = 0
    w0_ofs = negx_ofs + G * NT
    bb_ofs = w0_ofs + O
    wext_ofs = bb_ofs + O
    total = wext_ofs + G * (O + 1)
    return negx_ofs, w0_ofs, bb_ofs, wext_ofs, total


def _build(group_sizes: tuple[int, ...], d_mode: str, alphas: tuple[float, ...],
           repeats: int | None = None, loop_setup: bool = False):
    """group_sizes[g] channels share alphas[g]; density channel is the last
    channel of the last group. d_mode: 'derf' = single Derivative_Erf pass;
    'act' = Square pass + Exp pass (fallback). repeats wraps the compute body
    in a hardware loop (timing builds only)."""
    G = len(group_sizes)
    assert sum(group_sizes) == C
    negx_ofs, w0_ofs, bb_ofs, wext_ofs, FA = _aux_layout(G)
    nc = bacc.Bacc("TRN2", target_bir_lowering=False, debug=False)

    t_d = nc.dram_tensor("t_full", [M], F32, kind="ExternalInput")
    auxf_d = nc.dram_tensor("auxf", [P, FA], F32, kind="ExternalInput")
    FR = NT * C + (G * 2 * O + 512 if C_FP16 else 0)
    auxr_d = nc.dram_tensor("auxr", [P, FR], _e_dt(), kind="ExternalInput")
    out_d = nc.dram_tensor("out", [M, O], F32, kind="ExternalOutput")

    with tile.TileContext(nc) as tc:
        with (
            tc.tile_pool(name="singles", bufs=1) as singles,
            tc.tile_pool(name="work", bufs=2) as work,
            tc.tile_pool(name="psum", bufs=1, space=bass.MemorySpace.PSUM) as pp,
            tc.tile_pool(name="psum2", bufs=2, space=bass.MemorySpace.PSUM) as pp2,
        ):
            outer_cm = (tc.For_i(0, repeats, 1)
                        if (repeats and loop_setup) else contextlib.nullcontext())
            with outer_cm:
                if loop_setup:
                    singles = work
                _setup_and_body(nc, tc, group_sizes, d_mode, alphas, G,
                                t_d, auxf_d, auxr_d, out_d,
                                negx_ofs, w0_ofs, bb_ofs, wext_ofs, FA,
                                singles, work, pp, pp2,
                                repeats if not loop_setup else None)

    nc.compile()
    return nc


def _setup_and_body(nc, tc, group_sizes, d_mode, alphas, G,
                    t_d, auxf_d, auxr_d, out_d,
                    negx_ofs, w0_ofs, bb_ofs, wext_ofs, FA,
                    singles, work, pp, pp2, repeats):
    FR = NT * C + (G * 2 * O + 512 if C_FP16 else 0)
    if True:
        if True:
            # ---------------- setup (4 input DMAs total) ----------------
            t_b = singles.tile([P, M], F32)
            if T_MODE == "pbcast":
                # hybrid: half arrives via a parallel DMA broadcast on the
                # SP queue while gpsimd partition-broadcasts the other half
                t_row = singles.tile([1, M // 2], F32)
                t_in = t_d.ap()
                nc.sync.dma_start(
                    t_row,
                    bass.AP(tensor=t_in.tensor, offset=t_in.offset,
                            ap=[[0, 1], [1, M // 2]]),
                )
                nc.gpsimd.partition_broadcast(t_b[:, 0 : M // 2], t_row)
                nc.scalar.dma_start(
                    t_b[:, M // 2 : M],
                    bass.AP(tensor=t_in.tensor, offset=t_in.offset + M // 2,
                            ap=[[0, P], [1, M // 2]]),
                )
            else:
                t_ap = t_d.ap()
                half_ap = lambda h: bass.AP(
                    tensor=t_ap.tensor, offset=t_ap.offset + h * (M // 2),
                    ap=[[0, P], [1, M // 2]],
                )
                nc.sync.dma_start(t_b[:, 0 : M // 2], half_ap(0))
                nc.scalar.dma_start(t_b[:, M // 2 : M], half_ap(1))

            auxf = singles.tile([P, FA], F32)
            nc.gpsimd.dma_start(auxf, auxf_d.ap())
            auxr = singles.tile([P, FR], _e_dt())
            nc.gpsimd.dma_start(auxr, auxr_d.ap())
            ycat = auxr[:, 0 : NT * C].rearrange("p (a c) -> p a c", c=C)
            # (negx arrives with auxf before the first ACT pass; ycat only
            # gates the first matmul)

            negx = auxf[:, negx_ofs : negx_ofs + G * NT].rearrange(
                "p (g a) -> p g a", g=G
            )
            w0b = auxf[:, w0_ofs : w0_ofs + O].rearrange("p (u o) -> p u o", u=1)
            bb = auxf[:, bb_ofs : bb_ofs + O].rearrange("p (u o) -> p u o", u=1)
            wg_tiles = []
            wge_tiles = []
            if C_FP16:
                ones128 = singles.tile([1, P], _e_dt())
                nc.gpsimd.memset(ones128, 1.0)
                b_seed = auxr[0:1, NT * C + G * 2 * O : NT * C + G * 2 * O + 512]
            else:
                ones128 = b_seed = None
            for g, ng in enumerate(group_sizes):
                if C_FP16:
                    co = NT * C + g * 2 * O
                    wg_tiles.append(auxr[0:ng, co : co + 2 * O])
                    wge_tiles.append(None)
                else:
                    cofs = wext_ofs + g * (O + 1)
                    wg_tiles.append(auxf[0:ng, cofs : cofs + O + 1])
                    wge_tiles.append(None)

            loop_cm = tc.For_i(0, repeats, 1) if repeats else contextlib.nullcontext()
            with loop_cm:
                _bodyfn(nc, group_sizes, d_mode, alphas, G,
                        t_b, negx, ycat, wg_tiles, wge_tiles, w0b, bb,
                        ones128, b_seed, out_d, work, pp, pp2)


def _bodyfn(nc, group_sizes, d_mode, alphas, G,
            t_b, negx, ycat, wg_tiles, wge_tiles, w0b, bb,
            ones128, b_seed, out_d, work, pp, pp2):
    MH = M // 2          # m-half width
    JT = MH // P         # 8 m-tiles per half
    # ---------------- stages A+B ----------------
    yo_tiles = [[None, None] for _ in range(G)]   # [g][h] -> yo half tile
    if G == 1:
        psB = pp.tile([C, M], F32, tag="big")
    for g, ng in enumerate(group_sizes):
        if G > 1:
            psB = pp.tile([ng, M], F32, tag="big")
        sq_alpha = float(np.sqrt(alphas[g]))
        gofs = sum(group_sizes[:g])
        for i in range(NT):
            e = work.tile([P, M], _e_dt(), tag="e")
            if d_mode == "derf":
                nc.scalar.activation(
                    e, t_b, AF.Derivative_Erf,
                    bias=negx[:, g, i : i + 1], scale=sq_alpha,
                )
            else:
                d_sb = work.tile([P, M], F32, tag="dsb")
                nc.scalar.activation(
                    d_sb, t_b, AF.Square,
                    bias=negx[:, g, i : i + 1], scale=sq_alpha,
                )
                nc.scalar.activation(e, d_sb, AF.Exp, scale=-1.0)
            lhsT = ycat[:, i, gofs : gofs + ng] if G > 1 else ycat[:, i, :]
            for j in range(M // 512):
                csl = slice(j * 512, (j + 1) * 512)
                nc.tensor.matmul(
                    psB[0:ng, csl] if G > 1 else psB[:, csl],
                    lhsT,
                    e[:, csl],
                    start=(i == 0),
                    stop=(i == NT - 1),
                )
        if STAGES == "ab":
            continue
        for h in range(2):
            hsl = slice(h * MH, (h + 1) * MH)
            ngg = ng if G > 1 else C
            yo = work.tile([ngg, MH], _e_dt() if C_FP16 else F32,
                           tag=f"yo{g}h{h}")
            if h == 0:
                nc.scalar.copy(yo, psB[0:ngg, hsl])
            else:
                nc.vector.tensor_copy(yo, psB[0:ngg, hsl])
            yo_tiles[g][h] = yo
    if STAGES == "ab":
        # timing ablation: tiny consumer so stage B isn't dead code
        osb = work.tile([C, O], F32, tag="oab")
        nc.vector.tensor_copy(osb, psB[:, 0:O])
        nc.sync.dma_start(out_d.ap()[0:C, :], osb)
        return

    # ---------------- stage C (pipelined per m-half) ----------------
    # Within half h, m-tile jj covers global m = h*MH + 8p + jj, so each
    # partition's 8 output rows are DRAM-contiguous (2KB runs). Packed PSUM:
    # tile jj at cols [jj*128, jj*128+65) of a [P, MH] fp32 region (2 banks);
    # one zero region per 4 tiles (start only on the bank's first matmul).
    psE = None
    for h in range(2):
        psC = pp2.tile([P, JT, P], F32, tag="psc")
        yo_str = [yo_tiles[g][h].rearrange("c (p j) -> c j p", j=JT)
                  for g in range(G)]  # noqa
        if C_FP16:
            # seed each bank with the bias pattern ([0]*64 + b per tile,
            # b[o_star] zeroed), then accumulate merged [A'|E] matmuls
            for bank in range(2):
                nc.tensor.matmul(
                    psC[:, 4 * bank : 4 * bank + 4, :].rearrange(
                        "p j o -> p (j o)"),
                    ones128,
                    b_seed,
                    start=True,
                    stop=False,
                    skip_group_check=(bank > 0),
                )
            for jj in range(JT):
                for g, ng in enumerate(group_sizes):
                    nc.tensor.matmul(
                        psC[:, jj, :],
                        yo_str[g][:, jj, :],
                        wg_tiles[g],
                        start=False,
                        stop=(jj == JT - 1) and (g == G - 1),
                        skip_group_check=True,
                    )
        else:
            for jj in range(JT):
                for g, ng in enumerate(group_sizes):
                    first = (jj % 4 == 0) and (g == 0)
                    nc.tensor.matmul(
                        psC[:, jj, 0 : O + 1],
                        yo_str[g][:, jj, :],
                        wg_tiles[g],
                        start=first,
                        stop=(jj % 4 == 3) and (g == G - 1),
                        skip_group_check=not first,
                    )

        r = work.tile([P, JT, 1], F32, tag="r")
        osb = work.tile([P, JT, O], F32, tag="osb")
        if C_FP16:
            # r' = 1/E[o*] = 1/(D*W0[o*]); A was pre-scaled by W0[o*] so
            # osb = A'*r' + E  (E = b + D*W0, with b[o*] zeroed in the seed)
            nc.vector.reciprocal(r, psC[:, :, O + O_STAR[0] : O + O_STAR[0] + 1])
            nc.vector.tensor_mul(osb, psC[:, :, 0:O], r.to_broadcast((P, JT, O)))
            nc.vector.tensor_add(osb, osb, psC[:, :, O : 2 * O])
            if O_STAR[1] != 0.0:
                nc.vector.tensor_scalar_add(
                    osb[:, :, O_STAR[0] : O_STAR[0] + 1],
                    osb[:, :, O_STAR[0] : O_STAR[0] + 1], float(O_STAR[1]))
        elif True:
            # density >= O(10); the reference's +1e-8 is a bitwise no-op
            nc.vector.reciprocal(r, psC[:, :, O : O + 1])
            # osb = A * r
            nc.vector.tensor_mul(osb, psC[:, :, 0:O],
                                 r.to_broadcast((P, JT, O)))
            tmp = work.tile([P, JT, O], F32, tag="tmp")
            nc.vector.tensor_mul(
                tmp,
                psC[:, :, O : O + 1].to_broadcast((P, JT, O)),
                w0b.to_broadcast((P, JT, O)),
            )
            nc.vector.tensor_add(osb, osb, tmp)
            nc.vector.tensor_add(osb, osb, bb.to_broadcast((P, JT, O)))

        if STAGES == "abc":
            continue
        out_ap = out_d.ap()[h * MH : (h + 1) * MH, :].rearrange(
            "(p j) o -> p j o", j=JT
        )
        (nc.sync if h == 0 else nc.scalar).dma_start(out_ap, osb)
    if STAGES == "abc":
        osb2 = work.tile([P, O], F32, tag="oabc")
        nc.vector.tensor_copy(osb2, osb[:, 0, :])
        nc.sync.dma_start(out_d.ap()[0:P, :], osb2)


def _get_prog(group_sizes, alphas, d_mode):
    key = (tuple(group_sizes), tuple(np.float32(a) for a in alphas), d_mode,
           E_DTYPE, T_MODE, STAGES, C_FP16,
           (O_STAR[0], np.float32(O_STAR[1])) if C_FP16 else None)
    if key not in _PROG_CACHE:
        _PROG_CACHE[key] = _build(tuple(group_sizes), d_mode, tuple(alphas))
    return _PROG_CACHE[key]


D_MODE = "derf"
T_MODE = "pbcast"
STAGES = "all"
E_DTYPE = "fp16"
C_FP16 = True
O_STAR = [0, 0.0]
SQRT_PI_2 = float(np.sqrt(np.pi) / 2.0)


def _host_prep(x, y, t, sigma, W, b):
    """Returns (group_sizes, alphas, in_maps)."""
    scales = np.exp(sigma.astype(np.float64))
    alphas_all = 0.5 / (scales * scales)          # (9,)

    # group channels by identical alpha; density channel (0) goes last
    uniq = []
    for cidx in range(C):
        a = np.float32(alphas_all[cidx])
        for gu in uniq:
            if gu[0] == a:
                gu[1].append(cidx)
                break
        else:
            uniq.append([a, [cidx]])
    gi = next(i for i, gu in enumerate(uniq) if 0 in gu[1])
    uniq.append(uniq.pop(gi))
    uniq[-1][1].remove(0)
    uniq[-1][1].append(0)
    perm = [cidx for _, chans in uniq for cidx in chans]       # length 9
    group_sizes = tuple(len(chans) for _, chans in uniq)
    alphas = tuple(float(a) for a, _ in uniq)
    G = len(group_sizes)

    W_perm = W[perm, :]                                        # (9, 64)
    wext = np.zeros((C, O + 1), np.float32)
    wext[:CIN, :O] = W_perm[:CIN, :]
    wext[CIN, O] = 1.0                                         # density selector
    yperm_cols = [cidx - 1 for cidx in perm if cidx != 0]      # 8 y columns

    ysc = SQRT_PI_2 if D_MODE == "derf" else 1.0
    sq_alphas = np.sqrt(np.array(alphas, np.float64))
    negx_ofs, w0_ofs, bb_ofs, wext_ofs, FA = _aux_layout(G)

    in_maps = []
    for bi in range(B):
        auxf = np.zeros((P, FA), np.float32)
        # negx: [p, g, a] with n = a*128 + p
        negx = (-sq_alphas[:, None] * x[bi][None, :]).astype(np.float32)  # (G, N)
        auxf[:, negx_ofs : negx_ofs + G * NT] = (
            negx.reshape(G, NT, P).transpose(2, 0, 1).reshape(P, G * NT)
        )
        auxf[:, w0_ofs : w0_ofs + O] = W[0, :][None, :]
        auxf[:, bb_ofs : bb_ofs + O] = b[None, :]
        gofs = 0
        for g, ng in enumerate(group_sizes):
            cofs = wext_ofs + g * (O + 1)
            auxf[:ng, cofs : cofs + O + 1] = wext[gofs : gofs + ng, :]
            gofs += ng

        ycat_host = np.concatenate(
            [y[bi][:, yperm_cols], np.ones((N, 1), np.float32)], axis=1
        ) * np.float32(ysc)                                    # (N, 9)
        auxr = ycat_host.reshape(NT, P, C).transpose(1, 0, 2).reshape(P, NT * C)

        if C_FP16:
            W0 = W[0, :]
            ostar = int(np.argmax(np.abs(W0)))
            O_STAR[0] = ostar
            O_STAR[1] = float(b[ostar])
            ascale = np.float32(W0[ostar])
            blocks = [auxr]
            for g, ng in enumerate(group_sizes):
                wc = np.zeros((P, 2 * O), np.float32)
                gofs2 = sum(group_sizes[:g])
                # A' columns: conv weights scaled by W0[o*]; density row's
                # A-part is zero (wext col 0:64 of density row is 0)
                wc[:ng, 0:O] = wext[gofs2 : gofs2 + ng, 0:O] * ascale
                if g == len(group_sizes) - 1:
                    wc[ng - 1, O : 2 * O] = W0   # density row -> E = D*W0
                blocks.append(wc)
            bs = np.zeros((P, 512), np.float32)
            bpat = np.concatenate([np.zeros(O, np.float32), b.astype(np.float32)])
            bpat[O + ostar] = 0.0
            bs[0, :] = np.tile(bpat, 4)
            blocks.append(bs)
            auxr = np.concatenate(blocks, axis=1)
        if E_DTYPE == "bf16":
            import ml_dtypes
            auxr = auxr.astype(ml_dtypes.bfloat16)
        elif E_DTYPE == "fp16":
            auxr = auxr.astype(np.float16)
        else:
            auxr = auxr.astype(np.float32)
        in_maps.append({
            "t_full": np.ascontiguousarray(t[bi], np.float32),
            "auxf": np.ascontiguousarray(auxf),
            "auxr": np.ascontiguousarray(auxr),
        })
    return group_sizes, alphas, in_maps


def kernel(x, y, t, sigma, W, b):
    global C_FP16
    x = np.asarray(x, np.float32).reshape(B, N)
    y = np.asarray(y, np.float32).reshape(B, N, CIN)
    t = np.asarray(t, np.float32).reshape(B, M)
    sigma = np.asarray(sigma, np.float32).reshape(C)
    W = np.asarray(W, np.float32).reshape(C, O)
    b = np.asarray(b, np.float32).reshape(O)

    # the merged fp16 stage-C recovers 1/density from the D*W0[o*] column;
    # an all-zero W0 row would divide by zero there -> use the fp32 path
    if C_FP16 and abs(float(W[0, int(np.argmax(np.abs(W[0])))])) < 1e-30:
        C_FP16 = False

    group_sizes, alphas, in_maps = _host_prep(x, y, t, sigma, W, b)
    nc = _get_prog(group_sizes, alphas, D_MODE)

    res = run_bass_kernel_spmd(nc, in_maps, core_ids=list(range(B)))
    return np.stack([res.results[bi]["out"] for bi in range(B)], axis=0)


# revision 50
# speedup vs baseline: 1.1249x; 1.1249x over previous
"""ConvDeepSet Bass kernel for Trainium2 (8 NeuronCores, data-parallel over batch).

Math per batch b (reference):
    d[n,m]   = (x[n] - t[m])^2
    wt[n,m,c]= exp(-0.5 * d[n,m] / exp(sigma[c])^2) = exp(-alpha_c * d[n,m])
    ycat     = [ones, y]                       # (N, 9)
    yout[m,c]= sum_n ycat[n,c] * wt[n,m,c]     # (M, 9)
    h        = [yout[:,0], yout[:,1:]/(yout[:,0:1]+1e-8)]
    out      = h @ W + b                       # (M, 64)

Device mapping (one batch per core, n on partitions):
  Prologue: t broadcast to all 128 partitions via gpsimd partition_broadcast
           (much cheaper than a DMA broadcast); all small operands arrive
           packed in two host-prepared arrays (one fp32, one fp16) = 2 DMAs.
  Stage A+B ('derf' mode): ONE ACT pass per n-tile computes the Gaussian
           directly via Derivative_Erf(z) = (2/sqrt(pi)) * exp(-z^2) with
           z = sqrt(alpha)*t - sqrt(alpha)*x (free scale/bias of the
           activation); the sqrt(pi)/2 constant is folded into ycat
           host-side. The exp argument stays fp32 (precision-critical); the
           Gaussian values are emitted as fp16 (insensitive). The
           n-contraction runs as fp16 matmuls with ycat as lhsT accumulating
           y_out[9, M] in fp32 PSUM (channels permuted, density last).
  Stage C (per m-half, pipelined): y_out copied from PSUM to SBUF as fp16
           (one half on ACT, one on DVE). m-tile jj covers m = h*1024+8p+jj
           so each partition's 8 output rows are 2KB-contiguous in DRAM.
           Per tile ONE merged fp16 matmul with rhs [9, 128] =
           [W_conv*W0[o*] | E-block] writes [A' | E] into packed PSUM
           (4 tiles per 2KB bank, seeded per-bank with the bias pattern by a
           K=1 ones-row matmul). E = b + D*W0 with b[o*] zeroed, so
           r = 1/E[o*] recovers 1/(density*W0[o*]) and the DVE epilogue is
           just out = A'*r + E (reference's +1e-8 is a fp32 no-op since
           density is O(10)). One output DMA per half, on separate queues.

sigma is read on the host; channels are grouped by unique alpha (the
reference setup always produces a single group, the fast path; grouped
fallback handles arbitrary sigma). Degenerate all-zero W[0] falls back to a
full-fp32 stage C. Compiled programs are cached per grouping/config.

Measured (8-core SPMD, hardware loop differencing): ~25us per full kernel
iteration incl. setup DMAs; absmax/scale vs reference = 5.9e-4 (fp16
stage-C), 9.9e-5 with C_FP16=False.
"""

import contextlib

import numpy as np

import concourse.bass as bass
import concourse.mybir as mybir
import concourse.tile as tile
from concourse import bacc
from concourse.bass_utils import run_bass_kernel_spmd

B, N, M = 8, 512, 2048
JT0 = 8              # m-tiles per half (stage C)
CIN = 8
C = CIN + 1          # 9 channels incl. density
O = 64
P = 128
NT = N // P          # 4 n-tiles
MT = M // P          # 16 m-tiles for stage C
F32 = mybir.dt.float32
F32R = mybir.dt.float32r
BF16 = mybir.dt.bfloat16
FP16 = mybir.dt.float16
AF = mybir.ActivationFunctionType

_PROG_CACHE: dict = {}


def _e_dt():
    return {"bf16": BF16, "fp16": FP16}.get(E_DTYPE, F32R)


def _aux_layout(G):
    """Column offsets in the packed fp32 aux array: negx | w0b | bb | wext.
    The wext region holds G blocks of (O+1) columns, each block's rows at
    partitions [0:ng] (base partition 0 for every group's matmul rhs)."""
    negx_ofs = 0
    w0_ofs = negx_ofs + G * NT
    bb_ofs = w0_ofs + O
    wext_ofs = bb_ofs + O
    total = wext_ofs + G * (O + 1)
    return negx_ofs, w0_ofs, bb_ofs, wext_ofs, total


def _build(group_sizes: tuple[int, ...], d_mode: str, alphas: tuple[float, ...],
           repeats: int | None = None, loop_setup: bool = False):
    """group_sizes[g] channels share alphas[g]; density channel is the last
    channel of the last group. d_mode: 'derf' = single Derivative_Erf pass;
    'act' = Square pass + Exp pass (fallback). repeats wraps the compute body
    in a hardware loop (timing builds only)."""
    G = len(group_sizes)
    assert sum(group_sizes) == C
    negx_ofs, w0_ofs, bb_ofs, wext_ofs, FA = _aux_layout(G)
    nc = bacc.Bacc("TRN2", target_bir_lowering=False, debug=False)

    t_d = nc.dram_tensor("t_full", [M], F32, kind="ExternalInput")
    auxf_d = nc.dram_tensor("auxf", [P, FA], F32, kind="ExternalInput")
    FR = NT * C + (G * 2 * O + 512 if C_FP16 else 0)
    auxr_d = nc.dram_tensor("auxr", [P, FR], _e_dt(), kind="ExternalInput")
    out_d = nc.dram_tensor("out", [M, O], F32, kind="ExternalOutput")

    with tile.TileContext(nc) as tc:
        with (
            tc.tile_pool(name="singles", bufs=1) as singles,
            tc.tile_pool(name="work", bufs=2) as work,
            tc.tile_pool(name="psum", bufs=1, space=bass.MemorySpace.PSUM) as pp,
            tc.tile_pool(name="psum2", bufs=2, space=bass.MemorySpace.PSUM) as pp2,
        ):
            outer_cm = (tc.For_i(0, repeats, 1)
                        if (repeats and loop_setup) else contextlib.nullcontext())
            with outer_cm:
                if loop_setup:
                    singles = work
                _setup_and_body(nc, tc, group_sizes, d_mode, alphas, G,
                                t_d, auxf_d, auxr_d, out_d,
                                negx_ofs, w0_ofs, bb_ofs, wext_ofs, FA,
                                singles, work, pp, pp2,
                                repeats if not loop_setup else None)

    nc.compile()
    return nc


def _setup_and_body(nc, tc, group_sizes, d_mode, alphas, G,
                    t_d, auxf_d, auxr_d, out_d,
                    negx_ofs, w0_ofs, bb_ofs, wext_ofs, FA,
                    singles, work, pp, pp2, repeats):
    FR = NT * C + (G * 2 * O + 512 if C_FP16 else 0)
    if True:
        if True:
            # ---------------- setup (4 input DMAs total) ----------------
            t_b = singles.tile([P, M], F32)
            if T_MODE == "pbcast":
                t_row = singles.tile([1, M], F32)
                t_in = t_d.ap()
                nc.sync.dma_start(
                    t_row,
                    bass.AP(tensor=t_in.tensor, offset=t_in.offset,
                            ap=[[0, 1]] + list(t_in.ap)),
                )
                nc.gpsimd.partition_broadcast(t_b, t_row)
            else:
                t_ap = t_d.ap()
                half_ap = lambda h: bass.AP(
                    tensor=t_ap.tensor, offset=t_ap.offset + h * (M // 2),
                    ap=[[0, P], [1, M // 2]],
                )
                nc.sync.dma_start(t_b[:, 0 : M // 2], half_ap(0))
                nc.scalar.dma_start(t_b[:, M // 2 : M], half_ap(1))

            auxf = singles.tile([P, FA], F32)
            nc.sync.dma_start(auxf, auxf_d.ap())
            auxr = singles.tile([P, FR], _e_dt())
            nc.scalar.dma_start(auxr, auxr_d.ap())
            ycat = auxr[:, 0 : NT * C].rearrange("p (a c) -> p a c", c=C)
            # (negx arrives with auxf before the first ACT pass; ycat only
            # gates the first matmul)

            negx = auxf[:, negx_ofs : negx_ofs + G * NT].rearrange(
                "p (g a) -> p g a", g=G
            )
            w0b = auxf[:, w0_ofs : w0_ofs + O].rearrange("p (u o) -> p u o", u=1)
            bb = auxf[:, bb_ofs : bb_ofs + O].rearrange("p (u o) -> p u o", u=1)
            wg_tiles = []
            wge_tiles = []
            if C_FP16:
                ones128 = singles.tile([1, P], _e_dt())
                nc.gpsimd.memset(ones128, 1.0)
                b_seed = auxr[0:1, NT * C + G * 2 * O : NT * C + G * 2 * O + 512]
            else:
                ones128 = b_seed = None
            for g, ng in enumerate(group_sizes):
                if C_FP16:
                    co = NT * C + g * 2 * O
                    wg_tiles.append(auxr[0:ng, co : co + 2 * O])
                    wge_tiles.append(None)
                else:
                    cofs = wext_ofs + g * (O + 1)
                    wg_tiles.append(auxf[0:ng, cofs : cofs + O + 1])
                    wge_tiles.append(None)

            loop_cm = tc.For_i(0, repeats, 1) if repeats else contextlib.nullcontext()
            with loop_cm:
                _bodyfn(nc, group_sizes, d_mode, alphas, G,
                        t_b, negx, ycat, wg_tiles, wge_tiles, w0b, bb,
                        ones128, b_seed, out_d, work, pp, pp2)


def _bodyfn(nc, group_sizes, d_mode, alphas, G,
            t_b, negx, ycat, wg_tiles, wge_tiles, w0b, bb,
            ones128, b_seed, out_d, work, pp, pp2):
    MH = M // 2          # m-half width
    JT = MH // P         # 8 m-tiles per half
    # ---------------- stages A+B ----------------
    yo_tiles = [[None, None] for _ in range(G)]   # [g][h] -> yo half tile
    if G == 1:
        psB = pp.tile([C, M], F32, tag="big")
    for g, ng in enumerate(group_sizes):
        if G > 1:
            psB = pp.tile([ng, M], F32, tag="big")
        sq_alpha = float(np.sqrt(alphas[g]))
        gofs = sum(group_sizes[:g])
        for i in range(NT):
            e = work.tile([P, M], _e_dt(), tag="e")
            if d_mode == "derf":
                nc.scalar.activation(
                    e, t_b, AF.Derivative_Erf,
                    bias=negx[:, g, i : i + 1], scale=sq_alpha,
                )
            else:
                d_sb = work.tile([P, M], F32, tag="dsb")
                nc.scalar.activation(
                    d_sb, t_b, AF.Square,
                    bias=negx[:, g, i : i + 1], scale=sq_alpha,
                )
                nc.scalar.activation(e, d_sb, AF.Exp, scale=-1.0)
            lhsT = ycat[:, i, gofs : gofs + ng] if G > 1 else ycat[:, i, :]
            for j in range(M // 512):
                csl = slice(j * 512, (j + 1) * 512)
                nc.tensor.matmul(
                    psB[0:ng, csl] if G > 1 else psB[:, csl],
                    lhsT,
                    e[:, csl],
                    start=(i == 0),
                    stop=(i == NT - 1),
                )
        if STAGES == "ab":
            continue
        for h in range(2):
            hsl = slice(h * MH, (h + 1) * MH)
            ngg = ng if G > 1 else C
            yo = work.tile([ngg, MH], _e_dt() if C_FP16 else F32,
                           tag=f"yo{g}h{h}")
            if h == 0:
                nc.scalar.copy(yo, psB[0:ngg, hsl])
            else:
                nc.vector.tensor_copy(yo, psB[0:ngg, hsl])
            yo_tiles[g][h] = yo
    if STAGES == "ab":
        # timing ablation: tiny consumer so stage B isn't dead code
        osb = work.tile([C, O], F32, tag="oab")
        nc.vector.tensor_copy(osb, psB[:, 0:O])
        nc.sync.dma_start(out_d.ap()[0:C, :], osb)
        return

    # ---------------- stage C (pipelined per m-half) ----------------
    # Within half h, m-tile jj covers global m = h*MH + 8p + jj, so each
    # partition's 8 output rows are DRAM-contiguous (2KB runs). Packed PSUM:
    # tile jj at cols [jj*128, jj*128+65) of a [P, MH] fp32 region (2 banks);
    # one zero region per 4 tiles (start only on the bank's first matmul).
    psE = None
    for h in range(2):
        psC = pp2.tile([P, JT, P], F32, tag="psc")
        yo_str = [yo_tiles[g][h].rearrange("c (p j) -> c j p", j=JT)
                  for g in range(G)]  # noqa
        if C_FP16:
            # seed each bank with the bias pattern ([0]*64 + b per tile,
            # b[o_star] zeroed), then accumulate merged [A'|E] matmuls
            for bank in range(2):
                nc.tensor.matmul(
                    psC[:, 4 * bank : 4 * bank + 4, :].rearrange(
                        "p j o -> p (j o)"),
                    ones128,
                    b_seed,
                    start=True,
                    stop=False,
                    skip_group_check=(bank > 0),
                )
            for jj in range(JT):
                for g, ng in enumerate(group_sizes):
                    nc.tensor.matmul(
                        psC[:, jj, :],
                        yo_str[g][:, jj, :],
                        wg_tiles[g],
                        start=False,
                        stop=(jj == JT - 1) and (g == G - 1),
                        skip_group_check=True,
                    )
        else:
            for jj in range(JT):
                for g, ng in enumerate(group_sizes):
                    first = (jj % 4 == 0) and (g == 0)
                    nc.tensor.matmul(
                        psC[:, jj, 0 : O + 1],
                        yo_str[g][:, jj, :],
                        wg_tiles[g],
                        start=first,
                        stop=(jj % 4 == 3) and (g == G - 1),
                        skip_group_check=not first,
                    )

        r = work.tile([P, JT, 1], F32, tag="r")
        osb = work.tile([P, JT, O], F32, tag="osb")
        if C_FP16:
            # r' = 1/E[o*] = 1/(D*W0[o*]); A was pre-scaled by W0[o*] so
            # osb = A'*r' + E  (E = b + D*W0, with b[o*] zeroed in the seed)
            nc.vector.reciprocal(r, psC[:, :, O + O_STAR[0] : O + O_STAR[0] + 1])
            nc.vector.tensor_mul(osb, psC[:, :, 0:O], r.to_broadcast((P, JT, O)))
            nc.vector.tensor_add(osb, osb, psC[:, :, O : 2 * O])
            if O_STAR[1] != 0.0:
                nc.vector.tensor_scalar_add(
                    osb[:, :, O_STAR[0] : O_STAR[0] + 1],
                    osb[:, :, O_STAR[0] : O_STAR[0] + 1], float(O_STAR[1]))
        elif True:
            # density >= O(10); the reference's +1e-8 is a bitwise no-op
            nc.vector.reciprocal(r, psC[:, :, O : O + 1])
            # osb = A * r
            nc.vector.tensor_mul(osb, psC[:, :, 0:O],
                                 r.to_broadcast((P, JT, O)))
            tmp = work.tile([P, JT, O], F32, tag="tmp")
            nc.vector.tensor_mul(
                tmp,
                psC[:, :, O : O + 1].to_broadcast((P, JT, O)),
                w0b.to_broadcast((P, JT, O)),
            )
            nc.vector.tensor_add(osb, osb, tmp)
            nc.vector.tensor_add(osb, osb, bb.to_broadcast((P, JT, O)))

        if STAGES == "abc":
            continue
        out_ap = out_d.ap()[h * MH : (h + 1) * MH, :].rearrange(
            "(p j) o -> p j o", j=JT
        )
        (nc.sync if h == 0 else nc.scalar).dma_start(out_ap, osb)
    if STAGES == "abc":
        osb2 = work.tile([P, O], F32, tag="oabc")
        nc.vector.tensor_copy(osb2, osb[:, 0, :])
        nc.sync.dma_start(out_d.ap()[0:P, :], osb2)


def _get_prog(group_sizes, alphas, d_mode):
    key = (tuple(group_sizes), tuple(np.float32(a) for a in alphas), d_mode,
           E_DTYPE, T_MODE, STAGES, C_FP16,
           (O_STAR[0], np.float32(O_STAR[1])) if C_FP16 else None)
    if key not in _PROG_CACHE:
        _PROG_CACHE[key] = _build(tuple(group_sizes), d_mode, tuple(alphas))
    return _PROG_CACHE[key]


D_MODE = "derf"
T_MODE = "pbcast"
STAGES = "all"
E_DTYPE = "fp16"
C_FP16 = True
O_STAR = [0, 0.0]
SQRT_PI_2 = float(np.sqrt(np.pi) / 2.0)


def _host_prep(x, y, t, sigma, W, b):
    """Returns (group_sizes, alphas, in_maps)."""
    scales = np.exp(sigma.astype(np.float64))
    alphas_all = 0.5 / (scales * scales)          # (9,)

    # group channels by identical alpha; density channel (0) goes last
    uniq = []
    for cidx in range(C):
        a = np.float32(alphas_all[cidx])
        for gu in uniq:
            if gu[0] == a:
                gu[1].append(cidx)
                break
        else:
            uniq.append([a, [cidx]])
    gi = next(i for i, gu in enumerate(uniq) if 0 in gu[1])
    uniq.append(uniq.pop(gi))
    uniq[-1][1].remove(0)
    uniq[-1][1].append(0)
    perm = [cidx for _, chans in uniq for cidx in chans]       # length 9
    group_sizes = tuple(len(chans) for _, chans in uniq)
    alphas = tuple(float(a) for a, _ in uniq)
    G = len(group_sizes)

    W_perm = W[perm, :]                                        # (9, 64)
    wext = np.zeros((C, O + 1), np.float32)
    wext[:CIN, :O] = W_perm[:CIN, :]
    wext[CIN, O] = 1.0                                         # density selector
    yperm_cols = [cidx - 1 for cidx in perm if cidx != 0]      # 8 y columns

    ysc = SQRT_PI_2 if D_MODE == "derf" else 1.0
    sq_alphas = np.sqrt(np.array(alphas, np.float64))
    negx_ofs, w0_ofs, bb_ofs, wext_ofs, FA = _aux_layout(G)

    in_maps = []
    for bi in range(B):
        auxf = np.zeros((P, FA), np.float32)
        # negx: [p, g, a] with n = a*128 + p
        negx = (-sq_alphas[:, None] * x[bi][None, :]).astype(np.float32)  # (G, N)
        auxf[:, negx_ofs : negx_ofs + G * NT] = (
            negx.reshape(G, NT, P).transpose(2, 0, 1).reshape(P, G * NT)
        )
        auxf[:, w0_ofs : w0_ofs + O] = W[0, :][None, :]
        auxf[:, bb_ofs : bb_ofs + O] = b[None, :]
        gofs = 0
        for g, ng in enumerate(group_sizes):
            cofs = wext_ofs + g * (O + 1)
            auxf[:ng, cofs : cofs + O + 1] = wext[gofs : gofs + ng, :]
            gofs += ng

        ycat_host = np.concatenate(
            [y[bi][:, yperm_cols], np.ones((N, 1), np.float32)], axis=1
        ) * np.float32(ysc)                                    # (N, 9)
        auxr = ycat_host.reshape(NT, P, C).transpose(1, 0, 2).reshape(P, NT * C)

        if C_FP16:
            W0 = W[0, :]
            ostar = int(np.argmax(np.abs(W0)))
            O_STAR[0] = ostar
            O_STAR[1] = float(b[ostar])
            ascale = np.float32(W0[ostar])
            blocks = [auxr]
            for g, ng in enumerate(group_sizes):
                wc = np.zeros((P, 2 * O), np.float32)
                gofs2 = sum(group_sizes[:g])
                # A' columns: conv weights scaled by W0[o*]; density row's
                # A-part is zero (wext col 0:64 of density row is 0)
                wc[:ng, 0:O] = wext[gofs2 : gofs2 + ng, 0:O] * ascale
                if g == len(group_sizes) - 1:
                    wc[ng - 1, O : 2 * O] = W0   # density row -> E = D*W0
                blocks.append(wc)
            bs = np.zeros((P, 512), np.float32)
            bpat = np.concatenate([np.zeros(O, np.float32), b.astype(np.float32)])
            bpat[O + ostar] = 0.0
            bs[0, :] = np.tile(bpat, 4)
            blocks.append(bs)
            auxr = np.concatenate(blocks, axis=1)
        if E_DTYPE == "bf16":
            import ml_dtypes
            auxr = auxr.astype(ml_dtypes.bfloat16)
        elif E_DTYPE == "fp16":
            auxr = auxr.astype(np.float16)
        else:
            auxr = auxr.astype(np.float32)
        in_maps.append({
            "t_full": np.ascontiguousarray(t[bi], np.float32),
            "auxf": np.ascontiguousarray(auxf),
            "auxr": np.ascontiguousarray(auxr),
        })
    return group_sizes, alphas, in_maps


def kernel(x, y, t, sigma, W, b):
    global C_FP16
    x = np.asarray(x, np.float32).reshape(B, N)
    y = np.asarray(y, np.float32).reshape(B, N, CIN)
    t = np.asarray(t, np.float32).reshape(B, M)
    sigma = np.asarray(sigma, np.float32).reshape(C)
    W = np.asarray(W, np.float32).reshape(C, O)
    b = np.asarray(b, np.float32).reshape(O)

    # the merged fp16 stage-C recovers 1/density from the D*W0[o*] column;
    # an all-zero W0 row would divide by zero there -> use the fp32 path
    if C_FP16 and abs(float(W[0, int(np.argmax(np.abs(W[0])))])) < 1e-30:
        C_FP16 = False

    group_sizes, alphas, in_maps = _host_prep(x, y, t, sigma, W, b)
    nc = _get_prog(group_sizes, alphas, D_MODE)

    res = run_bass_kernel_spmd(nc, in_maps, core_ids=list(range(B)))
    return np.stack([res.results[bi]["out"] for bi in range(B)], axis=0)
